# revision 30
# baseline (speedup 1.0000x reference)
"""Trainium2 Bass kernel for nn_AUV_39565238730960.

Computation (per coil c, sharded 1 coil per NeuronCore over 8 cores):
    Z_b   = x_b * csm_c                 (complex elementwise, 30 images)
    Y_b   = T @ Z_b @ T                 (centered ortho 2D FFT as matmuls,
                                         T = symmetric centered DFT matrix)
    Yr    = interleave(Re Y, Im Y)      (30, 131072)
    out_c = mask * (VT^T @ Yr)          (200, 131072) -> stored fp16

v2 layout notes:
  - Output is written fp16 (tolerance budget allows it); host upcasts.
  - Yr never touches DRAM: per-image pass-R output is corner-turned with
    one SBUF->SBUF DMA per (image, kt) into resident strip tiles
    yr_kt[32*s + b, r*512 + c] (s = strip = p//32), so the projection's
    moving operand sits at 32-aligned partition bases -> the K=30
    matmuls row-tile across 4 PE strips on hardware.
  - Pass R is split by kt half: all images run pass L + pass R(kt=0)
    first (wt kept resident in fp16), so projection of the first 8
    n-groups overlaps with the remaining pass R(kt=1) work.
  - Masked PSUM eviction is split DVE / (Act copy + Pool mult) to
    balance engine time; everything lands as fp16 in SBUF before one
    contiguous fp16 DMA per 4096-column block.
"""

import numpy as np

NCH, NBASIS, NXS, NF = 8, 30, 256, 200
NX = NXS * NXS * 2

_CACHE = {}


def _fmat():
    """Symmetric centered orthonormal DFT matrix: fft1c(z) = T @ z."""
    eye = np.eye(NXS, dtype=np.complex128)
    t = np.fft.fftshift(
        np.fft.fft(np.fft.ifftshift(eye, axes=0), axis=0, norm="ortho"), axes=0
    )
    return t


def _build():
    import concourse.bacc as bacc
    import concourse.mybir as mybir
    import concourse.tile as tile

    F32 = mybir.dt.float32
    F32R = mybir.dt.float32r
    F16 = mybir.dt.float16
    U8 = mybir.dt.uint8
    MULT = mybir.AluOpType.mult
    ADD = mybir.AluOpType.add
    SUB = mybir.AluOpType.subtract

    t = _fmat()
    tr = t.real.astype(np.float32).reshape(2, 128, NXS).transpose(1, 0, 2)
    ti = t.imag.astype(np.float32).reshape(2, 128, NXS).transpose(1, 0, 2)
    # stacked moving operands (128, 2, 512): [Tr | Ti] and [-Ti | Tr]
    f_a = np.concatenate([tr, ti], axis=2).astype(np.float16)
    f_b = np.concatenate([-ti, tr], axis=2).astype(np.float16)

    nc = bacc.Bacc("TRN2", target_bir_lowering=False, debug=False, num_devices=NCH)

    # x / csm arrive with re/im planes separated: [.., 128, 2(rh), 2(reim), 256]
    x_d = nc.dram_tensor("x", [NBASIS, 128, 2, 2, NXS], F16, kind="ExternalInput")
    c_d = nc.dram_tensor("csm", [128, 2, 2, NXS], F16, kind="ExternalInput")
    v_d = nc.dram_tensor("vt", [NBASIS, NF], F32, kind="ExternalInput")
    m_d = nc.dram_tensor("mask", [NF, NX], U8, kind="ExternalInput")
    o_d = nc.dram_tensor("out", [NF, NX], F16, kind="ExternalOutput")

    fa_d = nc.inline_tensor(f_a, "fmat_a")
    fb_d = nc.inline_tensor(f_b, "fmat_b")

    MCHUNK = 4096  # mask / out tile width
    FT1_DVE = 144  # columns of each 512-wide ft1 psum evicted by DVE

    with tile.TileContext(nc) as tc:
        with (
            tc.tile_pool(name="const", bufs=1) as cpool,
            tc.tile_pool(name="work", bufs=1) as wpool,
            tc.tile_pool(name="psum", bufs=1, space="PSUM") as psum,
        ):
            # ---- constants ----
            fa = cpool.tile([128, 2, 512], F16, name="fa")
            fb = cpool.tile([128, 2, 512], F16, name="fb")
            nc.sync.dma_start(fa[:], fa_d.ap())
            nc.scalar.dma_start(fb[:], fb_d.ap())

            csm = cpool.tile([128, 2, 2, NXS], F16, name="csm")
            nc.gpsimd.dma_start(csm[:], c_d.ap())
            cr = csm[:, :, 0, :]
            ci = csm[:, :, 1, :]

            # VT replicated at partition bases 0/32/64/96 for row tiling
            vt32 = cpool.tile([128, NF], F32, name="vt32")
            for s in range(4):
                nc.gpsimd.dma_start(vt32[32 * s : 32 * s + NBASIS, :], v_d.ap())
            vt16 = cpool.tile([128, NF], F16, name="vt16")
            nc.vector.tensor_copy(vt16[:], vt32[:])

            # resident corner-turn destinations, one per kt half
            yr = [
                cpool.tile([128, 16384], F16, name=f"yr{kt}") for kt in range(2)
            ]
            # per-image pass-L output, fp16, kept until pass R kt=1 runs
            wt = [
                cpool.tile([128, 2, 512], F16, name=f"wt{b}") for b in range(NBASIS)
            ]

            def fft_pass_l(b):
                xb = wpool.tile([128, 2, 2, NXS], F16, name=f"xb{b}", tag="xb", bufs=4)
                nc.sync.dma_start(xb[:], x_d.ap()[b])
                xr = xb[:, :, 0, :]
                xi = xb[:, :, 1, :]

                ta = wpool.tile([128, 2, NXS], F16, name=f"ta{b}", tag="ta", bufs=2)
                tb = wpool.tile([128, 2, NXS], F16, name=f"tb{b}", tag="tb", bufs=2)
                nc.vector.tensor_tensor(ta[:], xr, cr, op=MULT)
                nc.vector.tensor_tensor(tb[:], xi, ci, op=MULT)
                zr = wpool.tile([128, 2, NXS], F16, name=f"zr{b}", tag="zr", bufs=2)
                nc.vector.tensor_tensor(zr[:], ta[:], tb[:], op=SUB)
                tc_ = wpool.tile([128, 2, NXS], F16, name=f"tc{b}", tag="ta", bufs=2)
                td = wpool.tile([128, 2, NXS], F16, name=f"td{b}", tag="tb", bufs=2)
                nc.vector.tensor_tensor(tc_[:], xr, ci, op=MULT)
                nc.vector.tensor_tensor(td[:], xi, cr, op=MULT)
                zi = wpool.tile([128, 2, NXS], F16, name=f"zi{b}", tag="zi", bufs=2)
                nc.vector.tensor_tensor(zi[:], tc_[:], td[:], op=ADD)

                # pass L: WT[j, k] = sum_i Z[i, j] T[i, k]   (W = T @ Z)
                for jt in range(2):
                    js = slice(jt * 128, (jt + 1) * 128)
                    pl = psum.tile([128, 512], F32, name=f"pl{b}_{jt}", tag="ps", bufs=8)
                    nc.tensor.matmul(pl[:], zr[:, 0, js], fa[:, 0, :], start=True, stop=False)
                    nc.tensor.matmul(pl[:], zr[:, 1, js], fa[:, 1, :], start=False, stop=False)
                    nc.tensor.matmul(pl[:], zi[:, 0, js], fb[:, 0, :], start=False, stop=False)
                    nc.tensor.matmul(pl[:], zi[:, 1, js], fb[:, 1, :], start=False, stop=True)
                    # alternate the jt=1 eviction between Act and DVE so
                    # neither exceeds the PE's per-image pace in phase A
                    if jt == 0 or b % 2 == 0:
                        nc.scalar.copy(wt[b][:, jt, :], pl[:])
                    else:
                        nc.vector.tensor_copy(wt[b][:, jt, :], pl[:])

            def fft_pass_r(b, kt):
                # pass R: Y[k, n] = sum_j WT[j, k] T[j, n]   (Y = W @ T)
                ksr = slice(kt * 128, (kt + 1) * 128)
                ksi = slice(256 + kt * 128, 256 + (kt + 1) * 128)
                pr = psum.tile([128, 512], F32, name=f"pr{b}_{kt}", tag="ps", bufs=8)
                nc.tensor.matmul(pr[:], wt[b][:, 0, ksr], fa[:, 0, :], start=True, stop=False)
                nc.tensor.matmul(pr[:], wt[b][:, 1, ksr], fa[:, 1, :], start=False, stop=False)
                nc.tensor.matmul(pr[:], wt[b][:, 0, ksi], fb[:, 0, :], start=False, stop=False)
                nc.tensor.matmul(pr[:], wt[b][:, 1, ksi], fb[:, 1, :], start=False, stop=True)
                # interleave re/im while evicting: y[.., c*2+ri] = pr[.., ri*256+c]
                yb = wpool.tile([128, 512], F16, name=f"yb{b}_{kt}", tag="yb", bufs=2)
                nc.scalar.copy(
                    yb[:].rearrange("p (c r) -> p c r", r=2),
                    pr[:].rearrange("p (r c) -> p c r", r=2),
                )
                # corner turn: yr[kt][32*(p//32) + b, (p%32)*512 + c] = yb[p, c]
                # Act queue: the corner DMA directly follows its producer
                # (the yb eviction) on the same queue, so it never waits at
                # the head, and HWDGE gen is ~0.65us vs ~1us on Pool
                nc.scalar.dma_start(yr[kt][b : b + 97 : 32, :], yb[:])

            def project(kt, s):
                # strip s of half kt covers n in [kt*65536 + s*16384, +16384)
                n0 = kt * 65536 + s * 16384
                ysl = yr[kt][32 * s : 32 * s + NBASIS, :]
                vt0 = vt16[32 * s : 32 * s + NBASIS, 0:128]
                vt1 = vt16[32 * s : 32 * s + NBASIS, 128:NF]
                pend = []
                for mc in range(16384 // MCHUNK):  # 4 mask/out blocks per strip
                    m0 = wpool.tile([128, MCHUNK], U8, name=f"m0_{kt}_{s}_{mc}", tag="m0", bufs=3)
                    m1 = wpool.tile([72, MCHUNK], U8, name=f"m1_{kt}_{s}_{mc}", tag="m1", bufs=2)
                    c0 = n0 + mc * MCHUNK
                    nc.sync.dma_start(m0[:], m_d.ap()[0:128, c0 : c0 + MCHUNK])
                    nc.sync.dma_start(m1[:], m_d.ap()[128:NF, c0 : c0 + MCHUNK])
                    ob0 = wpool.tile([128, MCHUNK], F16, name=f"ob0_{kt}_{s}_{mc}", tag="ob0", bufs=2)
                    ob1 = wpool.tile([72, MCHUNK], F16, name=f"ob1_{kt}_{s}_{mc}", tag="ob1", bufs=2)
                    for sub in range(MCHUNK // 1024):
                        off = mc * MCHUNK + sub * 1024
                        for h in range(2):
                            o2 = off + h * 512
                            s2h = slice(sub * 1024 + h * 512, sub * 1024 + (h + 1) * 512)
                            pp0 = psum.tile([128, 512], F32, name=f"pp0_{kt}_{s}_{mc}_{sub}_{h}", tag="ps", bufs=8)
                            nc.tensor.matmul(
                                pp0[:], vt0, ysl[:, o2 : o2 + 512],
                                start=True, stop=True, tile_position=(32 * s, 0),
                            )
                            nc.vector.tensor_tensor(ob0[:, s2h], pp0[:], m0[:, s2h], op=MULT)
                        for h in range(2):
                            o2 = off + h * 512
                            pp1 = psum.tile([72, 512], F32, name=f"pp1_{kt}_{s}_{mc}_{sub}_{h}", tag="ps", bufs=8)
                            nc.tensor.matmul(
                                pp1[:], vt1, ysl[:, o2 : o2 + 512],
                                start=True, stop=True, tile_position=(32 * s, 0),
                            )
                            # Act alone evicts pp1 (so the psum rotation never
                            # waits on the busier DVE/Pool queues); the masked
                            # multiply then runs from SBUF, split DVE / Pool
                            s2 = sub * 1024 + h * 512
                            stg = wpool.tile([72, 512], F16, name=f"stg_{kt}_{s}_{mc}_{sub}_{h}", tag="stg", bufs=4)
                            nc.scalar.copy(stg[:], pp1[:])
                            a_sl = slice(s2 + FT1_DVE, s2 + 512)
                            nc.gpsimd.tensor_tensor(ob1[:, a_sl], stg[:, FT1_DVE:512], m1[:, a_sl], op=MULT)
                            # defer the DVE share so it never head-blocks the
                            # ft0 eviction stream on DVE's in-order queue
                            d_sl = slice(s2, s2 + FT1_DVE)
                            pend.append((ob1, d_sl, stg, m1))
                            while len(pend) > 2:
                                o_, dsl_, st_, mm_ = pend.pop(0)
                                nc.vector.tensor_tensor(o_[:, dsl_], st_[:, 0:FT1_DVE], mm_[:, dsl_], op=MULT)
                    while pend:
                        o_, dsl_, st_, mm_ = pend.pop(0)
                        nc.vector.tensor_tensor(o_[:, dsl_], st_[:, 0:FT1_DVE], mm_[:, dsl_], op=MULT)
                    nc.sync.dma_start(o_d.ap()[0:128, c0 : c0 + MCHUNK], ob0[:])
                    nc.sync.dma_start(o_d.ap()[128:NF, c0 : c0 + MCHUNK], ob1[:])

            # ---- phase A: pass L + pass R(kt0) for all images ----
            for b in range(NBASIS):
                fft_pass_l(b)
                fft_pass_r(b, 0)
            # ---- phase B: pass R(kt1) interleaved with kt0 projection ----
            for s in range(4):
                for b in range(s * 8, min(NBASIS, (s + 1) * 8)):
                    fft_pass_r(b, 1)
                project(0, s)
            # ---- phase C: kt1 projection ----
            for s in range(4):
                project(1, s)

    nc.compile()
    return nc


def _get_nc():
    if "nc" not in _CACHE:
        _CACHE["nc"] = _build()
    return _CACHE["nc"]


def _prep_in_maps(x, csmT, VT, maskT):
    x = np.asarray(x, dtype=np.float32)
    # [b, r, col, reim] -> [b, p, rh, reim, col]  (r = rh*128 + p)
    x = np.ascontiguousarray(
        x.reshape(NBASIS, 2, 128, NXS, 2).transpose(0, 2, 1, 4, 3).astype(np.float16)
    )
    csm = np.asarray(csmT, dtype=np.float32)
    csm = np.ascontiguousarray(
        csm.reshape(NCH, 2, 128, NXS, 2).transpose(0, 2, 1, 4, 3).astype(np.float16)
    )
    vt = np.ascontiguousarray(np.asarray(VT, dtype=np.float32))
    mask = np.ascontiguousarray(np.asarray(maskT)).view(np.uint8)
    return [{"x": x, "csm": csm[c], "vt": vt, "mask": mask} for c in range(NCH)]


def kernel(x, csmT, VT, maskT):
    from concourse import bass2jax

    nc = _get_nc()
    in_maps = _prep_in_maps(x, csmT, VT, maskT)
    results = bass2jax.run_bass_via_pjrt(nc, in_maps, n_cores=NCH)
    return np.stack(
        [results[c]["out"].astype(np.float32) for c in range(NCH)], axis=0
    )
